# revision 4
# baseline (speedup 1.0000x reference)
"""Barycentric-coordinates kernel for Trainium2 (8 NeuronCores, data-parallel over V).

Device: pair-field scores + fused argmin per neighborhood (V2: custom fused
reciprocal ops, ACT-side relu/abs, 2-ra batching). Host: validated O(V*R*A)
finalization replicating the reference bit-exactly.
"""
import sys

for _p in ("/opt/trn_rl_repo", "/root/.axon_site/_ro/trn_rl_repo"):
    if _p not in sys.path:
        sys.path.append(_p)

import numpy as np

import concourse.bacc as bacc
import concourse.mybir as mybir
import concourse.tile as tile
from concourse import bass_utils

V, N, R, A = 1000, 32, 5, 8
RA = R * A
NCORES = 8
VS = V // NCORES
DELTA = 1e-30
RB = 2  # ra batch size

f32 = mybir.dt.float32
Alu = mybir.AluOpType
Act = mybir.ActivationFunctionType

_CACHE = {}


# --- custom DVE ops: r = C / (C^2 + delta) in two passes -------------------
def _register_custom_ops():
    if "ops" in _CACHE:
        return _CACHE["ops"]
    import concourse.dve_ops as dve_ops
    from concourse.dve_ops import DveOp, _COMPILE_CACHE
    from concourse.dve_spec import Spec, Src0, Src1, C0, C1, C2, Bin, AluOp, lower, sq
    from concourse.dve_uop import DveOpSpec

    fl = np.float32

    # pass 1 (in0=C): z = C*C + c2(delta); y0 = bitcast(~z)*c0; y1 = y0*(c1 - z*y0)
    z = sq(Src0) + C2
    noz = Bin(AluOp.BITWISE_NOT, z, z)
    y0 = noz * C0
    body1 = y0 * (C1 - z * y0)

    def ref1(in0, in1, c0, c1, c2):
        zz = (in0 * in0 + fl(c2)).astype(fl)
        not_z = (~zz.view(np.int32)).view(fl)
        yy0 = (not_z * fl(c0)).astype(fl)
        return (yy0 * (fl(c1) - zz * yy0).astype(fl)).astype(fl)

    # pass 2 (in0=y1, in1=C): z = C*C + c2; y2 = y1*(c1 - z*y1); r = y2*C
    z2 = sq(Src1) + C2
    y2 = Src0 * (C1 - z2 * Src0)
    body2 = y2 * Src1

    def ref2(in0, in1, c0, c1, c2):
        zz = (in1 * in1 + fl(c2)).astype(fl)
        yy2 = (in0 * (fl(c1) - zz * in0).astype(fl)).astype(fl)
        return (yy2 * in1).astype(fl)

    def make_op(name, body, ref):
        # compute the sha first, then pin it
        existing = [o for o in dve_ops.OPS if o.name == name]
        if existing:
            return existing[0]
        spec = Spec(body=body, reference=ref)
        row = dve_ops._CUSTOM_DVE_ROW_BASE + len(dve_ops.OPS)
        shas = {}
        for ver in ("v3", "v4"):
            try:
                u = lower(spec, ver=ver)
                shas[ver] = DveOpSpec(
                    name=name, opcode=row, uops=u, rd1_en=True
                ).sha(ver)
            except Exception:
                pass
        op = DveOp(name, spec, subdim=False, uops_sha=shas)
        dve_ops.OPS.append(op)
        dve_ops.CUSTOM_DVE_SPECS[name] = spec
        dve_ops._SUB_OPCODE_FOR_NAME[name] = row
        return op

    seed = make_op("ANT_RECIP_SQD_SEED", body1, ref1)
    fin = make_op("ANT_RECIP_SQD_FIN", body2, ref2)
    _CACHE["ops"] = (seed, fin)
    return seed, fin


# Chebyshev seed constants from RECIP_APPROX_FAST_CONSTS
_S0 = -0.23549792
_S1 = 2.0017324


def _build_nc():
    seed_op, fin_op = _register_custom_ops()
    nc = bacc.Bacc(
        "TRN2",
        target_bir_lowering=False,
        debug=False,
        enable_asserts=False,
        num_devices=NCORES,
    )
    proj_t = nc.dram_tensor("proj", [VS, N * 2], f32, kind="ExternalInput")
    templ_t = nc.dram_tensor("templ", [VS, RA * 2], f32, kind="ExternalInput")
    iota32_t = nc.dram_tensor("iota32", [VS, N], f32, kind="ExternalInput")
    piota_t = nc.dram_tensor("piota", [VS, N * N], f32, kind="ExternalInput")
    out_t = nc.dram_tensor("outp", [VS, 3 * RA], f32, kind="ExternalOutput")

    with tile.TileContext(nc) as tc:
        with (
            tc.tile_pool(name="const", bufs=1) as cpool,
            tc.tile_pool(name="p0w", bufs=1) as p0w,
            tc.tile_pool(name="p1w", bufs=1) as p1w,
        ):
            proj_sb = cpool.tile([VS, N * 2], f32)
            nc.sync.dma_start(proj_sb[:], proj_t.ap())
            t_sb = cpool.tile([VS, RA * 2], f32)
            nc.sync.dma_start(t_sb[:], templ_t.ap())
            iota32 = cpool.tile([VS, N], f32)
            nc.sync.dma_start(iota32[:], iota32_t.ap())
            piota = cpool.tile([VS, N * N], f32)
            nc.sync.dma_start(piota[:], piota_t.ap())

            pxy = proj_sb[:].rearrange("p (n c) -> p n c", c=2)
            px, py = pxy[:, :, 0], pxy[:, :, 1]
            px_b = px.unsqueeze(1).broadcast_to([VS, RA, N])
            py_b = py.unsqueeze(1).broadcast_to([VS, RA, N])
            txy = t_sb[:].rearrange("p (r c) -> p r c", c=2)
            tx, ty = txy[:, :, 0], txy[:, :, 1]
            tx_b = tx.unsqueeze(2).broadcast_to([VS, RA, N])
            ty_b = ty.unsqueeze(2).broadcast_to([VS, RA, N])
            iota_b = iota32[:].unsqueeze(1).broadcast_to([VS, RA, N])

            def t3(pool, tag):
                t = pool.tile([VS, RA * N], f32, tag=tag, name=tag)
                return t, t[:].rearrange("p (r n) -> p r n", n=N)

            TT = nc.vector.tensor_tensor
            TS = nc.vector.tensor_scalar

            dx_t, dx = t3(p0w, "dx")
            TT(out=dx, in0=px_b, in1=tx_b, op=Alu.subtract)
            dy_t, dy = t3(p0w, "dy")
            TT(out=dy, in0=py_b, in1=ty_b, op=Alu.subtract)
            sqx_t, sqx = t3(p0w, "sqx")
            TT(out=sqx, in0=dx, in1=dx, op=Alu.mult)
            sqy_t, sqy = t3(p0w, "sqy")
            TT(out=sqy, in0=dy, in1=dy, op=Alu.mult)
            d2_t, d2 = t3(p0w, "d2")
            TT(out=d2, in0=sqx, in1=sqy, op=Alu.add)

            dmin = p0w.tile([VS, RA], f32, tag="dmin", name="dmin")
            nc.vector.tensor_reduce(
                out=dmin[:], in_=d2, axis=mybir.AxisListType.X, op=Alu.min
            )
            dmin_b = dmin[:].unsqueeze(2).broadcast_to([VS, RA, N])
            oh_t, oh = t3(p0w, "oh")
            TT(out=oh, in0=d2, in1=dmin_b, op=Alu.is_equal)

            tmp_t, tmp = t3(p0w, "tmp")
            c_f = cpool.tile([VS, RA], f32)
            TT(out=tmp, in0=oh, in1=iota_b, op=Alu.mult)
            nc.vector.tensor_reduce(
                out=c_f[:], in_=tmp, axis=mybir.AxisListType.X, op=Alu.add
            )
            xc = p0w.tile([VS, RA], f32, tag="xc", name="xc")
            TT(out=tmp, in0=oh, in1=px_b, op=Alu.mult)
            nc.vector.tensor_reduce(
                out=xc[:], in_=tmp, axis=mybir.AxisListType.X, op=Alu.add
            )
            yc = p0w.tile([VS, RA], f32, tag="yc", name="yc")
            TT(out=tmp, in0=oh, in1=py_b, op=Alu.mult)
            nc.vector.tensor_reduce(
                out=yc[:], in_=tmp, axis=mybir.AxisListType.X, op=Alu.add
            )

            xc_b = xc[:].unsqueeze(2).broadcast_to([VS, RA, N])
            yc_b = yc[:].unsqueeze(2).broadcast_to([VS, RA, N])
            ux_t, ux = t3(cpool, "ux")
            TT(out=ux, in0=px_b, in1=xc_b, op=Alu.subtract)
            uy_t, uy = t3(cpool, "uy")
            TT(out=uy, in0=py_b, in1=yc_b, op=Alu.subtract)

            v2x = p0w.tile([VS, RA], f32, tag="v2x", name="v2x")
            TT(out=v2x[:], in0=tx, in1=xc[:], op=Alu.subtract)
            v2y = p0w.tile([VS, RA], f32, tag="v2y", name="v2y")
            TT(out=v2y[:], in0=ty, in1=yc[:], op=Alu.subtract)
            v2x_b = v2x[:].unsqueeze(2).broadcast_to([VS, RA, N])
            v2y_b = v2y[:].unsqueeze(2).broadcast_to([VS, RA, N])

            s1_t, s1 = t3(p0w, "s1")
            TT(out=s1, in0=uy, in1=v2x_b, op=Alu.mult)
            t2s_t, t2s = t3(p0w, "t2s")
            TT(out=t2s, in0=ux, in1=v2y_b, op=Alu.mult)
            ns_t, ns = t3(cpool, "ns")
            TT(out=ns, in0=t2s, in1=s1, op=Alu.subtract)
            s_t, s = t3(cpool, "s")
            TS(out=s, in0=ns, scalar1=-1.0, scalar2=None, op0=Alu.mult)

            ms_sb = cpool.tile([VS, RA], f32)
            rm_sb = cpool.tile([VS, RA], f32)

            ux3 = ux_t[:].rearrange("p (r n) -> p r n", n=N)
            uy3 = uy_t[:].rearrange("p (r n) -> p r n", n=N)
            s3 = s_t[:].rearrange("p (r n) -> p r n", n=N)
            ns3 = ns_t[:].rearrange("p (r n) -> p r n", n=N)

            FD = RB * N * N  # batched pair-field free size

            def btile(tag):
                t = p1w.tile([VS, FD], f32, tag=tag, name=tag)
                return t

            piota_bb = piota[:].unsqueeze(1).broadcast_to([VS, RB, N * N])

            for b0 in range(0, RA, RB):
                sl = slice(b0, b0 + RB)
                uxj = ux3[:, sl].unsqueeze(3).broadcast_to([VS, RB, N, N])
                uyj = uy3[:, sl].unsqueeze(3).broadcast_to([VS, RB, N, N])
                uxk = ux3[:, sl].unsqueeze(2).broadcast_to([VS, RB, N, N])
                uyk = uy3[:, sl].unsqueeze(2).broadcast_to([VS, RB, N, N])
                sk = s3[:, sl].unsqueeze(2).broadcast_to([VS, RB, N, N])
                nsj = ns3[:, sl].unsqueeze(3).broadcast_to([VS, RB, N, N])

                tA = btile("tA")
                tB = btile("tB")
                A4 = tA[:].rearrange("p (b j k) -> p b j k", b=RB, k=N)
                B4 = tB[:].rearrange("p (b j k) -> p b j k", b=RB, k=N)
                TT(out=A4, in0=uxj, in1=uyk, op=Alu.mult)       # t1 -> A
                TT(out=B4, in0=uyj, in1=uxk, op=Alu.mult)       # t2 -> B
                tC = btile("tC")
                TT(out=tC[:], in0=tA[:], in1=tB[:], op=Alu.subtract)  # C

                # r = C/(C^2+delta): 2 fused passes (seed into A, r into B)
                nc.vector._custom_dve(
                    seed_op, out=tA[:], in0=tC[:], s0=_S0, s1=_S1, imm2=DELTA
                )
                nc.vector._custom_dve(
                    fin_op, out=tB[:], in0=tA[:], in1=tC[:],
                    s0=0.0, s1=2.0, imm2=DELTA,
                )
                r4 = tB[:].rearrange("p (b j k) -> p b j k", b=RB, k=N)

                tP2 = btile("tP2")
                P24 = tP2[:].rearrange("p (b j k) -> p b j k", b=RB, k=N)
                TT(out=P24, in0=sk, in1=r4, op=Alu.mult)
                tP1 = btile("tP1")
                P14 = tP1[:].rearrange("p (b j k) -> p b j k", b=RB, k=N)
                TT(out=P14, in0=nsj, in1=r4, op=Alu.mult)
                tQ = btile("tQ")
                TT(out=tQ[:], in0=tP2[:], in1=tP1[:], op=Alu.add)

                # ACT: relu(-p0)=Relu(q-1), |p0|=Abs(1-q), plus p1/p2 variants
                tR0 = btile("tR0")
                nc.scalar.activation(tR0[:], tQ[:], Act.Relu, bias=-1.0, scale=1.0)
                tA0 = btile("tA0")
                nc.scalar.activation(tA0[:], tQ[:], Act.Abs, bias=1.0, scale=-1.0)
                tR1 = btile("tR1")
                nc.scalar.activation(tR1[:], tP1[:], Act.Relu, bias=0.0, scale=-1.0)
                tA1 = btile("tA1")
                nc.scalar.activation(tA1[:], tP1[:], Act.Abs, bias=0.0, scale=1.0)
                tR2 = btile("tR2")
                nc.scalar.activation(tR2[:], tP2[:], Act.Relu, bias=0.0, scale=-1.0)
                tA2 = btile("tA2")
                nc.scalar.activation(tA2[:], tP2[:], Act.Abs, bias=0.0, scale=1.0)

                TT(out=tA0[:], in0=tA0[:], in1=tA1[:], op=Alu.max)   # m01
                TT(out=tA0[:], in0=tA0[:], in1=tA2[:], op=Alu.max)   # m
                TT(out=tR0[:], in0=tR0[:], in1=tR1[:], op=Alu.add)   # sa
                TT(out=tR0[:], in0=tR0[:], in1=tR2[:], op=Alu.add)   # sb
                tSC = btile("tSC")
                TT(out=tSC[:], in0=tA0[:], in1=tR0[:], op=Alu.add)   # score

                sc3 = tSC[:].rearrange("p (b f) -> p b f", b=RB)
                nc.vector.tensor_reduce(
                    out=ms_sb[:, sl], in_=sc3, axis=mybir.AxisListType.X, op=Alu.min
                )
                tAM = btile("tAM")
                for i in range(RB):
                    ra = b0 + i
                    nc.vector.scalar_tensor_tensor(
                        out=tAM[:, i * N * N : (i + 1) * N * N],
                        in0=tSC[:, i * N * N : (i + 1) * N * N],
                        scalar=ms_sb[:, ra : ra + 1],
                        in1=piota[:],
                        op0=Alu.is_equal,
                        op1=Alu.mult,
                    )
                am3 = tAM[:].rearrange("p (b f) -> p b f", b=RB)
                nc.vector.tensor_reduce(
                    out=rm_sb[:, sl], in_=am3, axis=mybir.AxisListType.X, op=Alu.max
                )

            nc.sync.dma_start(out_t.ap()[:, 0:RA], c_f[:])
            nc.sync.dma_start(out_t.ap()[:, RA : 2 * RA], rm_sb[:])
            nc.sync.dma_start(out_t.ap()[:, 2 * RA : 3 * RA], ms_sb[:])

    nc.compile()
    return nc


def _get_nc():
    if "nc" not in _CACHE:
        _CACHE["nc"] = _build_nc()
    return _CACHE["nc"]


def _host_finalize(template, projections, c, fi, ms):
    """Validated f32 finalization replicating the reference's rounding exactly."""
    fl = np.float32
    f64 = np.float64
    T = template.reshape(RA, 2).astype(fl)
    px = projections[:, :, 0].astype(fl)
    py = projections[:, :, 1].astype(fl)
    vv = np.arange(V)[:, None] * np.ones((1, RA), dtype=np.int64)
    vv = vv.astype(np.int64)

    j = (fi // N).astype(np.int64)
    k = (fi % N).astype(np.int64)
    tx = T[None, :, 0]
    ty = T[None, :, 1]

    def at(arr, idx):
        return arr[vv, idx]

    pxc = at(px, c)
    pyc = at(py, c)

    def d2of(idx):
        ddx = (at(px, idx) - tx).astype(fl)
        ddy = (at(py, idx) - ty).astype(fl)
        return ((ddx * ddx).astype(fl) + (ddy * ddy).astype(fl)).astype(fl)

    d2j = d2of(j)
    d2k = d2of(k)
    k_closer = (d2k < d2j) | ((d2k == d2j) & (k < j))
    jc = np.where(k_closer, k, j)
    jf = np.where(k_closer, j, k)

    uxc = (at(px, jc) - pxc).astype(fl)
    uyc = (at(py, jc) - pyc).astype(fl)
    uxf = (at(px, jf) - pxc).astype(fl)
    uyf = (at(py, jf) - pyc).astype(fl)
    v2x = (tx - pxc).astype(fl)
    v2y = (ty - pyc).astype(fl)

    def fma(a, b, cc):
        return (f64(a) * f64(b) + f64(cc)).astype(fl)

    an = fma(uyc, uyc, (uxc * uxc).astype(fl))
    am = fma(uyf, uyf, (uxf * uxf).astype(fl))
    g = fma(uyc, uyf, (uxc * uxf).astype(fl))
    bn = fma(uyc, v2y, (uxc * v2x).astype(fl))
    bm = fma(uyf, v2y, (uxf * v2x).astype(fl))

    den = ((an * am).astype(fl) - (g * g).astype(fl)).astype(fl)
    den = np.where(den == 0.0, fl(1e-10), den)
    p2u = (((am * bn).astype(fl) - (g * bm).astype(fl)).astype(fl) / den).astype(fl)
    p1u = (((an * bm).astype(fl) - (g * bn).astype(fl)).astype(fl) / den).astype(fl)
    p0u = ((fl(1.0) - p2u).astype(fl) - p1u).astype(fl)
    p0l = ((fl(1.0) - p1u).astype(fl) - p2u).astype(fl)

    def relu(x):
        return np.maximum(-x, 0).astype(fl)

    su = (
        np.maximum(np.maximum(np.abs(p0u), np.abs(p2u)), np.abs(p1u)).astype(fl)
        + ((relu(p0u) + relu(p2u)).astype(fl) + relu(p1u)).astype(fl)
    ).astype(fl)
    sl = (
        np.maximum(np.maximum(np.abs(p0l), np.abs(p1u)), np.abs(p2u)).astype(fl)
        + ((relu(p0l) + relu(p1u)).astype(fl) + relu(p2u)).astype(fl)
    ).astype(fl)
    lower = sl < su

    i1 = np.where(lower, jf, jc)
    i2 = np.where(lower, jc, jf)
    w0 = np.where(lower, p0l, p0u)
    w1 = np.where(lower, p1u, p2u)
    w2 = np.where(lower, p2u, p1u)

    fb = ms >= fl(1.0)
    iw = np.stack(
        [
            np.where(fb, fl(1.0), w0),
            np.where(fb, fl(0.0), w1),
            np.where(fb, fl(0.0), w2),
        ],
        -1,
    ).astype(fl)
    idx = np.stack(
        [c, np.where(fb, c, i1), np.where(fb, c, i2)], -1
    ).astype(np.int32)
    return iw.reshape(V, R, A, 3), idx.reshape(V, R, A, 3)


def _device_run(template, projections, trace=False):
    nc = _get_nc()
    templ_flat = np.ascontiguousarray(
        template.reshape(1, RA * 2).astype(np.float32)
    ).repeat(VS, axis=0)
    iota32 = np.arange(N, dtype=np.float32)[None, :].repeat(VS, axis=0)
    piota = (N * N - np.arange(N * N, dtype=np.float32))[None, :].repeat(VS, axis=0)
    in_maps = []
    for cid in range(NCORES):
        shard = projections[cid * VS : (cid + 1) * VS].reshape(VS, N * 2)
        in_maps.append(
            {
                "proj": np.ascontiguousarray(shard.astype(np.float32)),
                "templ": templ_flat,
                "iota32": iota32,
                "piota": piota,
            }
        )
    res = bass_utils.run_bass_kernel_spmd(
        nc, in_maps, core_ids=list(range(NCORES)), trace=trace
    )
    outs = [res.results[cid]["outp"] for cid in range(NCORES)]
    full = np.concatenate(outs, axis=0)  # (V, 3*RA)
    c = np.rint(full[:, 0:RA]).astype(np.int64)
    fi = (N * N - np.rint(full[:, RA : 2 * RA])).astype(np.int64)
    ms = full[:, 2 * RA : 3 * RA].astype(np.float32)
    return c, fi, ms, res


def kernel(template, projections):
    template = np.asarray(template, dtype=np.float32)
    projections = np.asarray(projections, dtype=np.float32)
    c, fi, ms, _ = _device_run(template, projections)
    return _host_finalize(template, projections, c, fi, ms)
